# revision 34
# baseline (speedup 1.0000x reference)
"""Distributed GQA attention kernel for one TRN2 chip (8 NeuronCores).

Sharding: tensor-parallel over heads through attention, then an
AllToAll redistributes attention outputs so each core owns a token
slice and computes the FULL output projection (K=4096) locally --
no ReduceScatter of 33.5 MB wo partials (which cost a ~60us tail and
~95 MB of collective HBM traffic in the previous design). The A2A
moves only 4.2 MB per core, chunked 4x so it hides under attention
and the early wo chunks.

Core g owns query heads [4g, 4g+4) and kv head g. Attention blocks
are processed (qb outer, b inner); after both batches of qb finish,
a2a chunk qb fires. The wo phase (n-block outer, chunk inner) starts
after attention; chunk 3's wo comes last in each n-block, hiding the
final A2A's latency behind ~26us of wo MMs for chunks 0-2.

Layout choices (no on-device transposes of big activations):
  - x is passed pre-transposed (xT [D, B*L]) so projections contract D
    on the partition axis; x tb loads striped across the sync+vector
    HWDGE queues to feed phase 1's MM stream from t~=10us.
  - q/k are produced directly as qT/kT [head_dim, tokens] and stay
    resident in SBUF; scores are computed keys-on-partitions, so the
    P@V matmul consumes exp(scores) directly.
  - RoPE head_dim pairs are permuted (on the host, into wq/wk rows) so
    one DVE stream_shuffle does the rotation partner swap.
  - causal masking multiplies exp() by a constant 128x128 triangular
    0/1 tile on the DVE; no mask tensor on device.
  - softmax denominator: exp tiles accumulated on the DVE (bf16), one
    all-ones matmul per head broadcasts the partition sum; no max
    subtraction (fp32 logits here are <~15).
  - A2A shard j rows = [head h=0..3][hd d=0..127] for j's tokens, so
    the gathered buffer rows are exactly wo's input dims in natural
    order: row j*512+h*128+d = global head (4j+h), dim d. The wo GEMM
    consumes the gathered tiles as lhsT with zero reshuffling.
"""

import numpy as np

import concourse.bass as bass
import concourse.mybir as mybir
import concourse.tile as tile
from concourse import bacc
from concourse.alu_op_type import AluOpType
from concourse.masks import make_identity, make_upper_triangular

F32 = mybir.dt.float32
BF16 = mybir.dt.bfloat16

N_CORES = 8
NHL = 4           # local q heads per core
HD = 128          # head dim
THETA = 10000.0
SCALE = HD ** -0.5
TW = 512          # token block width (free dim of most matmuls)
KW = 128          # key tile width (partition dim of score tiles)
KC = N_CORES * NHL  # 32 gathered head-chunks = wo contraction blocks

# module-level knobs for test.py
TRACE = False
LAST_RESULTS = None


class Cfg:
    def __init__(self, B=2, L=2048, D=4096):
        self.B, self.L, self.D = B, L, D
        assert B == 2
        self.BL = B * L
        self.DC = D // 128         # contraction chunks for projections
        self.NB = L // TW          # query blocks per batch
        self.NT = self.BL // TW    # token blocks total
        self.KT = L // KW          # key tiles per batch
        self.NBLK = D // TW        # wo output column blocks
        self.NCH = self.NB         # a2a chunks: chunk qb = both batches
        self.SH = self.B * TW // N_CORES   # a2a shard tokens (=128)
        assert self.SH == 128 and self.BL % TW == 0


# stream_shuffle mask: swap 16-partition halves within each 32-partition quadrant
SWAP16 = [(i + 16) % 32 for i in range(32)]


def _rope_perm():
    """Permutation of head_dim rows: pair i=(16q + r) lives at partitions
    32q+r (x1 = even dim 2i) and 32q+16+r (x2 = odd dim 2i+1)."""
    perm = np.zeros(HD, dtype=np.int64)
    for p in range(HD):
        q, r = divmod(p, 32)
        i = 16 * q + (r % 16)
        perm[p] = 2 * i + (0 if r < 16 else 1)
    return perm


def _rope_tables(cfg):
    """cosT/sinT [128, L] in the permuted-partition layout, sin sign-folded."""
    t = np.arange(cfg.L, dtype=np.float64)
    freqs = THETA ** (-np.arange(0, HD, 2, dtype=np.float64) / HD)  # [64]
    theta = t[None, :] * freqs[:, None]                             # [64, L]
    cos, sin = np.cos(theta), np.sin(theta)
    C = np.zeros((HD, cfg.L), dtype=np.float32)
    S = np.zeros((HD, cfg.L), dtype=np.float32)
    for p in range(HD):
        q, r = divmod(p, 32)
        i = 16 * q + (r % 16)
        C[p] = cos[i]
        S[p] = sin[i] if r >= 16 else -sin[i]
    return C, S


def classify_mask(mask, cfg):
    """cls[kt][qb] = (kind, off): kind in {'Z','N','M'} for tile
    mask[qb*TW:(qb+1)*TW, kt*KW:(kt+1)*KW]; off = count of leading query
    columns in the tile that are fully masked (safe to skip: exp would
    be exactly 0 there). M tiles must match the causal staircase -- the
    device applies them with a constant triangular multiply."""
    cls = [[None] * cfg.NB for _ in range(cfg.KT)]
    for kt in range(cfg.KT):
        for qb in range(cfg.NB):
            t = mask[qb * TW:(qb + 1) * TW, kt * KW:(kt + 1) * KW]
            if np.all(t == 0.0):
                cls[kt][qb] = ('Z', 0)
            elif np.all(t <= -1e8):
                cls[kt][qb] = ('N', 0)
            else:
                qq = np.arange(qb * TW, (qb + 1) * TW)[:, None]
                kk = np.arange(kt * KW, (kt + 1) * KW)[None, :]
                causal = kk <= qq
                assert np.all((t == 0.0) == causal) and \
                    np.all(t[~causal] <= -1e8), \
                    "partial mask tiles must be causal"
                dead_q = np.all(t <= -1e8, axis=1)  # [TW]
                off = 0
                while off < len(dead_q) and dead_q[off]:
                    off += 1
                off = (off // 64) * 64  # keep offsets 64-aligned
                cls[kt][qb] = ('M', off)
    # guard: every query block must attend to at least one key tile
    for qb in range(cfg.NB):
        assert any(cls[kt][qb][0] != 'N' for kt in range(cfg.KT)), \
            "fully-masked query block unsupported"
    return cls


def build_bass(cfg, cls):
    nc = bacc.Bacc("TRN2", target_bir_lowering=False, debug=False,
                   num_devices=N_CORES, num_swdge_queues=4)

    xT_d = nc.dram_tensor("xT", [cfg.D, cfg.BL], BF16, kind="ExternalInput")
    wqT_d = nc.dram_tensor("wqT", [cfg.D, NHL * HD], BF16, kind="ExternalInput")
    wkT_d = nc.dram_tensor("wkT", [cfg.D, HD], BF16, kind="ExternalInput")
    wvT_d = nc.dram_tensor("wvT", [cfg.D, HD], BF16, kind="ExternalInput")
    woT_d = nc.dram_tensor("woT", [KC * HD, cfg.D], BF16, kind="ExternalInput")
    ropeC_d = nc.dram_tensor("ropeC", [HD, cfg.L], F32, kind="ExternalInput")
    ropeS_d = nc.dram_tensor("ropeS", [HD, cfg.L], F32, kind="ExternalInput")
    out_d = nc.dram_tensor("out", [cfg.NCH * cfg.SH, cfg.D], BF16,
                           kind="ExternalOutput")

    rg = [list(range(N_CORES))]
    QD = NHL * HD  # 512

    with tile.TileContext(nc) as tc:
        # ---- constants ----------------------------------------------------
        const_pool = tc.alloc_tile_pool(name="const", bufs=1)
        ones_sb = const_pool.tile([128, 128], BF16, name="ones_sb")
        nc.vector.memset(ones_sb[:], 1.0)
        ident = const_pool.tile([128, 128], BF16, name="ident")
        make_identity(nc, ident[:])
        # causal staircase: tri[p, u] = 1 iff u >= p (keep key p on query u)
        tri_sb = const_pool.tile([128, 128], BF16, name="tri_sb")
        make_upper_triangular(nc, tri_sb[:], val=1.0, diag=True)

        # ---- resident activations -----------------------------------------
        kv_pool = tc.alloc_tile_pool(name="kv", bufs=1)
        kT_sb = kv_pool.tile([HD, cfg.BL], BF16, name="kT_sb")
        v_sb = kv_pool.tile([128, cfg.BL], BF16, name="v_sb")
        q_pool = tc.alloc_tile_pool(name="qres", bufs=1)
        qT_sb = q_pool.tile([HD, NHL * cfg.BL], BF16, name="qT_sb")

        # DRAM scratch: a2a staging per chunk
        dram_pool = tc.alloc_tile_pool(name="dram", bufs=1, space="DRAM")
        warm_in = dram_pool.tile([N_CORES, 1024], BF16, name="warm_in")
        warm_out = dram_pool.tile([N_CORES, 1024], BF16, name="warm_out")
        a2a_in = [dram_pool.tile([KC * HD, cfg.SH], BF16, name=f"a2a_in{i}")
                  for i in range(cfg.NCH)]
        a2a_out = [dram_pool.tile([KC * HD, cfg.SH], BF16, name=f"a2a_out{i}")
                   for i in range(cfg.NCH)]

        # ---- weights: all released after phase 1 (wo streams in phase 2b)
        rtbl_pool = tc.alloc_tile_pool(name="ropetbl", bufs=1)
        ropeC = rtbl_pool.tile([HD, cfg.L], F32, name="ropeC_sb")
        ropeS = rtbl_pool.tile([HD, cfg.L], F32, name="ropeS_sb")
        rtmp_pool = tc.alloc_tile_pool(name="ropetmp", bufs=3)
        vst_pool = tc.alloc_tile_pool(name="vstage", bufs=2)
        x_pool = tc.alloc_tile_pool(name="xload", bufs=12)
        w_pool = tc.alloc_tile_pool(name="weights", bufs=1)
        wq_sb = w_pool.tile([128, cfg.DC * QD], BF16, name="wq_sb")
        wk_sb = w_pool.tile([128, cfg.DC * HD], BF16, name="wk_sb")
        wv_sb = w_pool.tile([128, cfg.DC * HD], BF16, name="wv_sb")

        def load_w3d(eng, dst, src_d, width, chunk, interleave=None):
            """dst[:, dc*width+c] = src[dc*128+p, c], batched `chunk` dcs/DMA.
            With interleave=(dst2, src2): alternate chunks of two tensors."""
            for d0 in range(0, cfg.DC, chunk):
                d1 = min(d0 + chunk, cfg.DC)
                for dd, ss in ((dst, src_d),) + (interleave or ()):
                    eng.dma_start(
                        out=dd[:, d0 * width:d1 * width]
                        .rearrange("p (dc c) -> p dc c", dc=d1 - d0),
                        in_=ss.ap()[d0 * 128:d1 * 128, :]
                        .rearrange("(dc p) c -> p dc c", p=128))

        # wk/wv on the SWDGE queue (gpsimd) so they don't delay x on sync;
        # interleaved so the first dc chunks of BOTH land early.
        load_w3d(nc.gpsimd, wk_sb, wkT_d, HD, 8, interleave=((wv_sb, wvT_d),))
        # wq head-major (1MB per head) after tb0's odd x chunks on scalar:
        # head h's block is complete before the h-outer q loop reaches it
        for h in range(NHL):
            nc.scalar.dma_start(
                out=wq_sb[:, h * cfg.DC * HD:(h + 1) * cfg.DC * HD]
                .rearrange("p (dc c) -> p dc c", dc=cfg.DC),
                in_=wqT_d.ap()[:, h * HD:(h + 1) * HD]
                .rearrange("(dc p) c -> p dc c", p=128))
        nc.gpsimd.dma_start(out=ropeC[:], in_=ropeC_d.ap())
        nc.gpsimd.dma_start(out=ropeS[:], in_=ropeS_d.ap())
        # prime the mesh collective path with a tiny AllToAll so the first
        # real one doesn't pay the slow-start (runs under phase 1)
        wz = vst_pool.tile([N_CORES, 1024], BF16, name="warm_z")
        nc.vector.memset(wz[:], 0.0)
        nc.scalar.dma_start(out=warm_in[:], in_=wz[:])
        nc.gpsimd.collective_compute(
            "AllToAll", AluOpType.bypass, replica_groups=rg,
            ins=[warm_in[:].opt()], outs=[warm_out[:].opt()])

        # ================= phase 1: QKV projections + RoPE =================
        # q: one PSUM tile per head (h-outer MM loop + per-head rope drain
        # right after that head's MMs) so banks free progressively and the
        # last tb's drains never gate phase 2a's PSUM pools.
        with tc.tile_pool(name="qpsum", bufs=4, space="PSUM") as q_psum, \
             tc.tile_pool(name="kpsum", bufs=1, space="PSUM") as k_psum, \
             tc.tile_pool(name="vpsum", bufs=1, space="PSUM") as v_psum, \
             tc.tile_pool(name="vtpsum", bufs=1, space="PSUM") as vt_psum:

            def rope_drain(ps, dst):
                """dst = ps*C + shuffle16(ps)*S (tables sliced at t0)."""
                sw = rtmp_pool.tile([128, TW], F32, name="rope_sw")
                t1 = rtmp_pool.tile([128, TW], F32, name="rope_t1")
                t2 = rtmp_pool.tile([128, TW], F32, name="rope_t2")
                nc.vector.stream_shuffle(sw[:], ps, SWAP16)
                nc.vector.tensor_tensor(t1[:], sw[:], Sx, AluOpType.mult)
                nc.vector.tensor_tensor(t2[:], ps, Cx, AluOpType.mult)
                nc.vector.tensor_tensor(dst, t1[:], t2[:], AluOpType.add)

            for tb in range(cfg.NT):
                t0 = (tb % cfg.NB) * TW  # position within batch
                Cx = ropeC[:, t0:t0 + TW]
                Sx = ropeS[:, t0:t0 + TW]

                k_ps = k_psum.tile([128, TW], F32, name="k_ps")
                vT_ps = v_psum.tile([128, TW], F32, name="vT_ps")
                xts = []
                XB = 2 if tb < 2 else 4  # dc-chunks per DMA (small first)
                for ci, dc in enumerate(range(0, cfg.DC, XB)):
                    d1 = min(dc + XB, cfg.DC)
                    xt = x_pool.tile([128, (d1 - dc) * TW], BF16, name="x_t")
                    nc.sync.dma_start(
                        out=xt[:].rearrange("p (dc t) -> p dc t", dc=d1 - dc),
                        in_=xT_d.ap()[dc * 128:d1 * 128,
                                      tb * TW:(tb + 1) * TW]
                        .rearrange("(dc p) t -> p dc t", p=128))
                    for j in range(d1 - dc):
                        xts.append(xt[:, j * TW:(j + 1) * TW])
                for dc in range(cfg.DC):
                    st = dict(start=(dc == 0), stop=(dc == cfg.DC - 1))
                    nc.tensor.matmul(k_ps[:],
                                     wk_sb[:, dc * HD:(dc + 1) * HD],
                                     xts[dc], **st)
                    nc.tensor.matmul(vT_ps[:],
                                     wv_sb[:, dc * HD:(dc + 1) * HD],
                                     xts[dc], **st)
                # k: rope drain on DVE runs under the q MMs below
                rope_drain(k_ps[:], kT_sb[:, tb * TW:(tb + 1) * TW])
                vt_sb = vst_pool.tile([128, TW], BF16, name="vT_stage")
                nc.scalar.copy(vt_sb[:], vT_ps[:])
                for h in range(NHL):
                    q_ps = q_psum.tile([128, TW], F32, name="q_ps")
                    for dc in range(cfg.DC):
                        nc.tensor.matmul(
                            q_ps[:],
                            wq_sb[:, (h * cfg.DC + dc) * HD:
                                  (h * cfg.DC + dc + 1) * HD],
                            xts[dc], start=(dc == 0), stop=(dc == cfg.DC - 1))
                    rope_drain(q_ps[:],
                               qT_sb[:, h * cfg.BL + tb * TW:
                                     h * cfg.BL + (tb + 1) * TW])
                    if h == 0:
                        # v transposes tucked after h0's MMs: the vT stage
                        # copy has certainly landed by then
                        for i in range(TW // 128):
                            vp = vt_psum.tile([128, 128], BF16, name="v_tr_ps")
                            nc.tensor.transpose(vp[:],
                                                vt_sb[:, i * 128:(i + 1) * 128],
                                                ident[:])
                            nc.scalar.copy(
                                v_sb[:, tb * TW + i * 128:
                                     tb * TW + (i + 1) * 128],
                                vp[:])

        w_pool.release()
        x_pool.release()
        vst_pool.release()
        rtmp_pool.release()
        rtbl_pool.release()

        # ============ phase 2a: attention + a2a staging ====================
        # attnS_sb[c]: DMA-landing tile for gathered chunk c. The wo
        # matmuls read attnT_sb[c], produced by a DVE copy from attnS:
        # DMA-completion semaphores on the dynamic queues resolve
        # unpredictably late (sub-queue rotation / SWDGE drain), engine
        # semaphores are exact.
        at2_pool = tc.alloc_tile_pool(name="attnT", bufs=1)
        attnS_sb = [at2_pool.tile([128, KC * cfg.SH], BF16, name=f"attnS{c}")
                    for c in range(cfg.NCH)]
        attnT_sb = [at2_pool.tile([128, KC * cfg.SH], BF16, name=f"attnT{c}")
                    for c in range(cfg.NCH)]
        # wo weight stream: one n-block resident + one loading; preload
        # n=0 now so phase 2b starts without a DMA stall. Loads ride the
        # gpsimd SWDGE, chunked 4 kc per DMA: a single big DIRECT2D on a
        # compute engine stream would stall it for ~10us.
        wo_pool = tc.alloc_tile_pool(name="wo_w", bufs=2)
        woSB_t = {}

        def load_wo_nblock(n, eng):
            # ONE DMA per n-block (not chunked): a single completion
            # semaphore means the wo ldweights carries the attnT wait and
            # the matmul the woSB wait -- no event-semaphore, whose
            # wait-merging otherwise hoists late-a2a waits to the front
            # of the wo phase. n0 rides sync (before any staging); the
            # rest ride scalar, issued in phase 2b after all the exps.
            # (SWDGE mis-executes this large strided descriptor -> NaNs.)
            woSB = wo_pool.tile([128, KC * TW], BF16, name="wo_nb")
            eng.dma_start(
                out=woSB[:].rearrange("p (kc n) -> p kc n", kc=KC),
                in_=woT_d.ap()[:, n * TW:(n + 1) * TW]
                .rearrange("(kc p) n -> p kc n", p=128))
            woSB_t[n] = woSB

        load_wo_nblock(0, nc.scalar)

        # one pool scope for attention + wo: a scope boundary would
        # barrier phase 2b behind the last a2a staging DMA. Attention
        # PSUM pools sit on the right side (banks freed earliest by
        # phase 1); o_psum on the left.
        with tc.tile_pool(name="expsb", bufs=4) as e_pool, \
             tc.tile_pool(name="attnsb", bufs=2) as at_pool, \
             tc.tile_pool(name="recsb", bufs=2) as rec_pool, \
             tc.tile_pool(name="ostage", bufs=4) as oc_pool, \
             tc.tile_pool(name="scps", bufs=3, space="PSUM",
                          side="right") as sc_psum, \
             tc.tile_pool(name="avps", bufs=2, space="PSUM",
                          side="right") as av_psum, \
             tc.tile_pool(name="seps", bufs=1, space="PSUM",
                          side="right") as se_psum, \
             tc.tile_pool(name="ops", bufs=2, space="PSUM") as o_psum:

            for qb in range(cfg.NB):
                active = [kt for kt in range(cfg.KT)
                          if cls[kt][qb][0] != 'N']
                offs = {kt: cls[kt][qb][1] for kt in active}
                offs[active[0]] = 0

                for b in range(cfg.B):
                    attn_sb = at_pool.tile([128, NHL * TW], BF16, name="at_sb")
                    tb2 = b * cfg.NB + qb
                    # flattened (head, key-tile) sequence with the score
                    # lookahead crossing head boundaries, so the pipeline
                    # never restarts at a head start
                    LOOKAHEAD = 2
                    n_act = len(active)
                    seq = [(h, j) for h in range(NHL) for j in range(n_act)]
                    sc_tiles = {}

                    def qt_of(h):
                        return qT_sb[:, h * cfg.BL + tb2 * TW:
                                     h * cfg.BL + (tb2 + 1) * TW]

                    def emit_sc(h, j):
                        kt2 = active[j]
                        gk2 = b * cfg.L + kt2 * KW
                        o = offs[kt2]
                        sc = sc_psum.tile([KW, TW], F32, name="sc_ps")
                        nc.tensor.matmul(sc[:, o:], kT_sb[:, gk2:gk2 + KW],
                                         qt_of(h)[:, o:],
                                         start=True, stop=True)
                        sc_tiles[(h, j)] = sc

                    for p in range(min(LOOKAHEAD, len(seq))):
                        emit_sc(*seq[p])
                    head_state = {}
                    for pos, (h, idx) in enumerate(seq):
                        if pos + LOOKAHEAD < len(seq):
                            emit_sc(*seq[pos + LOOKAHEAD])
                        if idx == 0:
                            head_state[h] = (
                                av_psum.tile([HD, TW], F32, name="at_ps"),
                                e_pool.tile([KW, TW], BF16, name="esum",
                                            bufs=2))
                        at_ps, esum = head_state[h]
                        kt = active[idx]
                        gk = b * cfg.L + kt * KW  # global key token
                        o = offs[kt]
                        sc_ps = sc_tiles.pop((h, idx))
                        ex = e_pool.tile([KW, TW], BF16, name="ex_t")
                        nc.scalar.activation(
                            ex[:, o:], sc_ps[:, o:],
                            mybir.ActivationFunctionType.Exp,
                            scale=float(SCALE))
                        if cls[kt][qb][0] == 'M':
                            # zero the masked staircase, which for
                            # 128-aligned tiles is exactly the 128
                            # query columns starting at w0: key p is
                            # kept on window column u iff u >= p.
                            w0 = kt * KW - qb * TW
                            nc.vector.tensor_tensor(
                                ex[:, w0:w0 + KW], ex[:, w0:w0 + KW],
                                tri_sb[:], AluOpType.mult)
                        # denominator: accumulate exp tiles on the DVE;
                        # one ones-matmul at the end broadcasts the sum
                        if idx == 0:
                            nc.vector.tensor_copy(esum[:], ex[:])
                        else:
                            nc.vector.tensor_tensor(
                                esum[:, o:], esum[:, o:], ex[:, o:],
                                AluOpType.add)
                        st = dict(start=(idx == 0), stop=(idx == n_act - 1))
                        nc.tensor.matmul(at_ps[:, o:], v_sb[:, gk:gk + KW],
                                         ex[:, o:], **st)
                        if idx == n_act - 1:  # head epilogue
                            se_ps = se_psum.tile([128, TW], F32, name="se_ps")
                            nc.tensor.matmul(se_ps[:], ones_sb[:], esum[:],
                                             start=True, stop=True)
                            rec = rec_pool.tile([128, TW], F32, name="rec_t")
                            nc.vector.reciprocal_approx_fast(rec[:], se_ps[:])
                            # attn_sb columns are (j, h, t): j = 128-token
                            # group, so the a2a staging DMA is one
                            # contiguous 3-dim copy per block
                            nc.vector.tensor_tensor(
                                attn_sb[:]
                                .rearrange("p (j r) -> p j r", j=TW // 128)
                                [:, :, h * 128:(h + 1) * 128],
                                at_ps[:].rearrange("p (j t) -> p j t",
                                                   j=TW // 128),
                                rec[:].rearrange("p (j t) -> p j t",
                                                 j=TW // 128),
                                AluOpType.mult)
                            del head_state[h]

                    # stage this block into a2a_in[qb]: shard (b*4+jj) rows
                    # [h*128+d] for tokens jj*128..  of this block
                    nc.sync.dma_start(
                        out=a2a_in[qb][b * (TW // 128) * QD:
                                       (b + 1) * (TW // 128) * QD, :]
                        .rearrange("(r p) t -> p r t", p=128),
                        in_=attn_sb[:].rearrange("p (r t) -> p r t",
                                                 r=(TW // 128) * NHL))
                    if b == cfg.B - 1:
                        nc.gpsimd.collective_compute(
                            "AllToAll", AluOpType.bypass, replica_groups=rg,
                            ins=[a2a_in[qb][:].opt()],
                            outs=[a2a_out[qb][:].opt()])
                        # land gathered chunk into SBUF for the wo phase
                        nc.sync.dma_start(
                            out=attnS_sb[qb][:]
                            .rearrange("p (kc t) -> p kc t", kc=KC),
                            in_=a2a_out[qb][:]
                            .rearrange("(kc p) t -> p kc t", p=128))

            # ---- phase 2b: wo = attnT.T @ woT, token-sliced ---------------
            # launder the gathered chunks through the DVE: wo matmuls then
            # wait on exact Vector-engine semaphore values
            for c in range(cfg.NCH):
                nc.vector.tensor_copy(attnT_sb[c][:], attnS_sb[c][:])

            # The LAST chunk's MM group for each n-block is deferred by one
            # n-stage: the compiler's wait-merging otherwise hoists the
            # "attnT[last] loaded" (= last a2a done) wait to the front of
            # the whole wo phase, stalling the PE ~30us.
            def wo_group(n, c, woSB):
                o_ps = o_psum.tile([128, TW], F32, name="o_ps")
                for kc in range(KC):
                    nc.tensor.matmul(
                        o_ps[:],
                        attnT_sb[c][:, kc * cfg.SH:(kc + 1) * cfg.SH],
                        woSB[:, kc * TW:(kc + 1) * TW],
                        start=(kc == 0), stop=(kc == KC - 1))
                oc = oc_pool.tile([128, TW], BF16, name="oc_t")
                if c % 2 == 0:
                    nc.vector.tensor_copy(oc[:], o_ps[:])
                else:
                    nc.scalar.copy(oc[:], o_ps[:])
                nc.sync.dma_start(
                    out=out_d.ap()[c * cfg.SH:(c + 1) * cfg.SH,
                                   n * TW:(n + 1) * TW],
                    in_=oc[:])

            cl = cfg.NCH - 1  # the a2a3-gated chunk
            for n in range(cfg.NBLK):
                if n >= 1:
                    # previous stage's deferred chunk first: frees the wo
                    # slot the upcoming load reuses
                    wo_group(n - 1, cl, woSB_t.pop(n - 1))
                if n + 1 < cfg.NBLK:
                    load_wo_nblock(n + 1, nc.scalar)
                for c in range(cl):
                    wo_group(n, c, woSB_t[n])
            wo_group(cfg.NBLK - 1, cl, woSB_t.pop(cfg.NBLK - 1))

        wo_pool.release()
        at2_pool.release()
        dram_pool.release()
        q_pool.release()
        kv_pool.release()
        const_pool.release()

    nc.compile()
    return nc


def host_prepare(cfg, x, mask, wq, wk, wv, wo):
    """Returns (in_maps, cls)."""
    x = np.ascontiguousarray(np.asarray(x, dtype=np.float32))
    mask = np.asarray(mask, dtype=np.float32)
    wq = np.asarray(wq, dtype=np.float32)
    wk = np.asarray(wk, dtype=np.float32)
    wv = np.asarray(wv, dtype=np.float32)
    wo = np.asarray(wo, dtype=np.float32)

    import ml_dtypes
    bf16 = ml_dtypes.bfloat16
    perm = _rope_perm()
    C, S = _rope_tables(cfg)
    xT = np.ascontiguousarray(x.reshape(cfg.BL, cfg.D).T).astype(bf16)
    cls = classify_mask(mask, cfg)
    # full wo, transposed: row (4j+h)*128+d matches the a2a row order
    woT = np.ascontiguousarray(wo.T).astype(bf16)

    in_maps = []
    for g in range(N_CORES):
        qrows = wq[g * NHL * HD:(g + 1) * NHL * HD]          # [512, D]
        qperm = np.concatenate(
            [qrows[h * HD + perm] for h in range(NHL)], axis=0)
        krows = wk[g * HD:(g + 1) * HD][perm]                # [128, D]
        vrows = wv[g * HD:(g + 1) * HD]                      # [128, D]
        in_maps.append({
            "xT": xT,
            "wqT": np.ascontiguousarray(qperm.T).astype(bf16),
            "wkT": np.ascontiguousarray(krows.T).astype(bf16),
            "wvT": np.ascontiguousarray(vrows.T).astype(bf16),
            "woT": woT,
            "ropeC": C,
            "ropeS": S,
        })
    return in_maps, cls


def assemble_output(cfg, results):
    """Stitch per-core token slices back into [B, L, D].
    Core g's out row c*128+p = batch (g<4 ? 0 : 1), token qb=c*TW +
    (g%4)*128 + p."""
    full = np.empty((cfg.BL, cfg.D), dtype=np.float32)
    for g in range(N_CORES):
        r = np.asarray(results[g]["out"]).astype(np.float32)
        b = g // (TW // 128)
        jj = g % (TW // 128)
        for c in range(cfg.NCH):
            tok = b * cfg.L + c * TW + jj * 128
            full[tok:tok + 128] = r[c * cfg.SH:(c + 1) * cfg.SH]
    return full.reshape(cfg.B, cfg.L, cfg.D)


def kernel(x, mask, wq, wk, wv, wo):
    global LAST_RESULTS
    from concourse.bass_utils import run_bass_kernel_spmd
    cfg = Cfg(B=2, L=2048, D=4096)
    in_maps, cls = host_prepare(cfg, x, mask, wq, wk, wv, wo)
    nc = build_bass(cfg, cls)
    res = run_bass_kernel_spmd(nc, in_maps, core_ids=list(range(N_CORES)),
                               trace=TRACE)
    LAST_RESULTS = res
    return assemble_output(cfg, res.results)


# revision 37
# speedup vs baseline: 1.0008x; 1.0008x over previous
"""Distributed GQA attention kernel for one TRN2 chip (8 NeuronCores).

Sharding: tensor-parallel over heads through attention, then an
AllToAll redistributes attention outputs so each core owns a token
slice and computes the FULL output projection (K=4096) locally --
no ReduceScatter of 33.5 MB wo partials (which cost a ~60us tail and
~95 MB of collective HBM traffic in the previous design). The A2A
moves only 4.2 MB per core, chunked 4x so it hides under attention
and the early wo chunks.

Core g owns query heads [4g, 4g+4) and kv head g. Attention blocks
are processed (qb outer, b inner); after both batches of qb finish,
a2a chunk qb fires. The wo phase (n-block outer, chunk inner) starts
after attention; chunk 3's wo comes last in each n-block, hiding the
final A2A's latency behind ~26us of wo MMs for chunks 0-2.

Layout choices (no on-device transposes of big activations):
  - x is passed pre-transposed (xT [D, B*L]) so projections contract D
    on the partition axis; x tb loads striped across the sync+vector
    HWDGE queues to feed phase 1's MM stream from t~=10us.
  - q/k are produced directly as qT/kT [head_dim, tokens] and stay
    resident in SBUF; scores are computed keys-on-partitions, so the
    P@V matmul consumes exp(scores) directly.
  - RoPE head_dim pairs are permuted (on the host, into wq/wk rows) so
    one DVE stream_shuffle does the rotation partner swap.
  - causal masking multiplies exp() by a constant 128x128 triangular
    0/1 tile on the DVE; no mask tensor on device.
  - softmax denominator: exp tiles accumulated on the DVE (bf16), one
    all-ones matmul per head broadcasts the partition sum; no max
    subtraction (fp32 logits here are <~15).
  - A2A shard j rows = [head h=0..3][hd d=0..127] for j's tokens, so
    the gathered buffer rows are exactly wo's input dims in natural
    order: row j*512+h*128+d = global head (4j+h), dim d. The wo GEMM
    consumes the gathered tiles as lhsT with zero reshuffling.
"""

import numpy as np

import concourse.bass as bass
import concourse.mybir as mybir
import concourse.tile as tile
from concourse import bacc
from concourse.alu_op_type import AluOpType
from concourse.masks import make_identity, make_upper_triangular

F32 = mybir.dt.float32
BF16 = mybir.dt.bfloat16

N_CORES = 8
NHL = 4           # local q heads per core
HD = 128          # head dim
THETA = 10000.0
SCALE = HD ** -0.5
TW = 512          # token block width (free dim of most matmuls)
KW = 128          # key tile width (partition dim of score tiles)
KC = N_CORES * NHL  # 32 gathered head-chunks = wo contraction blocks

# module-level knobs for test.py
TRACE = False
LAST_RESULTS = None


class Cfg:
    def __init__(self, B=2, L=2048, D=4096):
        self.B, self.L, self.D = B, L, D
        assert B == 2
        self.BL = B * L
        self.DC = D // 128         # contraction chunks for projections
        self.NB = L // TW          # query blocks per batch
        self.NT = self.BL // TW    # token blocks total
        self.KT = L // KW          # key tiles per batch
        self.NBLK = D // TW        # wo output column blocks
        self.NCH = self.NB         # a2a chunks: chunk qb = both batches
        self.SH = self.B * TW // N_CORES   # a2a shard tokens (=128)
        assert self.SH == 128 and self.BL % TW == 0


# stream_shuffle mask: swap 16-partition halves within each 32-partition quadrant
SWAP16 = [(i + 16) % 32 for i in range(32)]


def _rope_perm():
    """Permutation of head_dim rows: pair i=(16q + r) lives at partitions
    32q+r (x1 = even dim 2i) and 32q+16+r (x2 = odd dim 2i+1)."""
    perm = np.zeros(HD, dtype=np.int64)
    for p in range(HD):
        q, r = divmod(p, 32)
        i = 16 * q + (r % 16)
        perm[p] = 2 * i + (0 if r < 16 else 1)
    return perm


def _rope_tables(cfg):
    """cosT/sinT [128, L] in the permuted-partition layout, sin sign-folded."""
    t = np.arange(cfg.L, dtype=np.float64)
    freqs = THETA ** (-np.arange(0, HD, 2, dtype=np.float64) / HD)  # [64]
    theta = t[None, :] * freqs[:, None]                             # [64, L]
    cos, sin = np.cos(theta), np.sin(theta)
    C = np.zeros((HD, cfg.L), dtype=np.float32)
    S = np.zeros((HD, cfg.L), dtype=np.float32)
    for p in range(HD):
        q, r = divmod(p, 32)
        i = 16 * q + (r % 16)
        C[p] = cos[i]
        S[p] = sin[i] if r >= 16 else -sin[i]
    return C, S


def classify_mask(mask, cfg):
    """cls[kt][qb] = (kind, off): kind in {'Z','N','M'} for tile
    mask[qb*TW:(qb+1)*TW, kt*KW:(kt+1)*KW]; off = count of leading query
    columns in the tile that are fully masked (safe to skip: exp would
    be exactly 0 there). M tiles must match the causal staircase -- the
    device applies them with a constant triangular multiply."""
    cls = [[None] * cfg.NB for _ in range(cfg.KT)]
    for kt in range(cfg.KT):
        for qb in range(cfg.NB):
            t = mask[qb * TW:(qb + 1) * TW, kt * KW:(kt + 1) * KW]
            if np.all(t == 0.0):
                cls[kt][qb] = ('Z', 0)
            elif np.all(t <= -1e8):
                cls[kt][qb] = ('N', 0)
            else:
                qq = np.arange(qb * TW, (qb + 1) * TW)[:, None]
                kk = np.arange(kt * KW, (kt + 1) * KW)[None, :]
                causal = kk <= qq
                assert np.all((t == 0.0) == causal) and \
                    np.all(t[~causal] <= -1e8), \
                    "partial mask tiles must be causal"
                dead_q = np.all(t <= -1e8, axis=1)  # [TW]
                off = 0
                while off < len(dead_q) and dead_q[off]:
                    off += 1
                off = (off // 64) * 64  # keep offsets 64-aligned
                cls[kt][qb] = ('M', off)
    # guard: every query block must attend to at least one key tile
    for qb in range(cfg.NB):
        assert any(cls[kt][qb][0] != 'N' for kt in range(cfg.KT)), \
            "fully-masked query block unsupported"
    return cls


def build_bass(cfg, cls):
    nc = bacc.Bacc("TRN2", target_bir_lowering=False, debug=False,
                   num_devices=N_CORES, num_swdge_queues=4)

    xT_d = nc.dram_tensor("xT", [cfg.D, cfg.BL], BF16, kind="ExternalInput")
    wqT_d = nc.dram_tensor("wqT", [cfg.D, NHL * HD], BF16, kind="ExternalInput")
    wkT_d = nc.dram_tensor("wkT", [cfg.D, HD], BF16, kind="ExternalInput")
    wvT_d = nc.dram_tensor("wvT", [cfg.D, HD], BF16, kind="ExternalInput")
    woT_d = nc.dram_tensor("woT", [KC * HD, cfg.D], BF16, kind="ExternalInput")
    ropeC_d = nc.dram_tensor("ropeC", [HD, cfg.L], F32, kind="ExternalInput")
    ropeS_d = nc.dram_tensor("ropeS", [HD, cfg.L], F32, kind="ExternalInput")
    out_d = nc.dram_tensor("out", [cfg.NCH * cfg.SH, cfg.D], BF16,
                           kind="ExternalOutput")

    rg = [list(range(N_CORES))]
    QD = NHL * HD  # 512

    with tile.TileContext(nc) as tc:
        # ---- constants ----------------------------------------------------
        const_pool = tc.alloc_tile_pool(name="const", bufs=1)
        ones_sb = const_pool.tile([128, 128], BF16, name="ones_sb")
        nc.vector.memset(ones_sb[:], 1.0)
        ident = const_pool.tile([128, 128], BF16, name="ident")
        make_identity(nc, ident[:])
        # causal staircase: tri[p, u] = 1 iff u >= p (keep key p on query u)
        tri_sb = const_pool.tile([128, 128], BF16, name="tri_sb")
        make_upper_triangular(nc, tri_sb[:], val=1.0, diag=True)

        # ---- resident activations -----------------------------------------
        kv_pool = tc.alloc_tile_pool(name="kv", bufs=1)
        kT_sb = kv_pool.tile([HD, cfg.BL], BF16, name="kT_sb")
        v_sb = kv_pool.tile([128, cfg.BL], BF16, name="v_sb")
        q_pool = tc.alloc_tile_pool(name="qres", bufs=1)
        qT_sb = q_pool.tile([HD, NHL * cfg.BL], BF16, name="qT_sb")

        # DRAM scratch: a2a staging per chunk
        dram_pool = tc.alloc_tile_pool(name="dram", bufs=1, space="DRAM")
        warm_in = dram_pool.tile([N_CORES, 1024], BF16, name="warm_in")
        warm_out = dram_pool.tile([N_CORES, 1024], BF16, name="warm_out")
        a2a_in = [dram_pool.tile([KC * HD, cfg.SH], BF16, name=f"a2a_in{i}")
                  for i in range(cfg.NCH)]
        a2a_out = [dram_pool.tile([KC * HD, cfg.SH], BF16, name=f"a2a_out{i}")
                   for i in range(cfg.NCH)]

        # ---- weights: all released after phase 1 (wo streams in phase 2b)
        rtbl_pool = tc.alloc_tile_pool(name="ropetbl", bufs=1)
        ropeC = rtbl_pool.tile([HD, cfg.L], F32, name="ropeC_sb")
        ropeS = rtbl_pool.tile([HD, cfg.L], F32, name="ropeS_sb")
        rtmp_pool = tc.alloc_tile_pool(name="ropetmp", bufs=3)
        vst_pool = tc.alloc_tile_pool(name="vstage", bufs=2)
        x_pool = tc.alloc_tile_pool(name="xload", bufs=12)
        w_pool = tc.alloc_tile_pool(name="weights", bufs=1)
        wq_sb = w_pool.tile([128, cfg.DC * QD], BF16, name="wq_sb")
        wk_sb = w_pool.tile([128, cfg.DC * HD], BF16, name="wk_sb")
        wv_sb = w_pool.tile([128, cfg.DC * HD], BF16, name="wv_sb")

        def load_w3d(eng, dst, src_d, width, chunk, interleave=None):
            """dst[:, dc*width+c] = src[dc*128+p, c], batched `chunk` dcs/DMA.
            With interleave=(dst2, src2): alternate chunks of two tensors."""
            for d0 in range(0, cfg.DC, chunk):
                d1 = min(d0 + chunk, cfg.DC)
                for dd, ss in ((dst, src_d),) + (interleave or ()):
                    eng.dma_start(
                        out=dd[:, d0 * width:d1 * width]
                        .rearrange("p (dc c) -> p dc c", dc=d1 - d0),
                        in_=ss.ap()[d0 * 128:d1 * 128, :]
                        .rearrange("(dc p) c -> p dc c", p=128))

        # wk/wv on the SWDGE queue (gpsimd) so they don't delay x on sync;
        # interleaved so the first dc chunks of BOTH land early.
        load_w3d(nc.gpsimd, wk_sb, wkT_d, HD, 8, interleave=((wv_sb, wvT_d),))
        # wq head-major (1MB per head) after tb0's odd x chunks on scalar:
        # head h's block is complete before the h-outer q loop reaches it
        for h in range(NHL):
            nc.scalar.dma_start(
                out=wq_sb[:, h * cfg.DC * HD:(h + 1) * cfg.DC * HD]
                .rearrange("p (dc c) -> p dc c", dc=cfg.DC),
                in_=wqT_d.ap()[:, h * HD:(h + 1) * HD]
                .rearrange("(dc p) c -> p dc c", p=128))
        nc.gpsimd.dma_start(out=ropeC[:], in_=ropeC_d.ap())
        nc.gpsimd.dma_start(out=ropeS[:], in_=ropeS_d.ap())
        # prime the mesh collective path with a tiny AllToAll so the first
        # real one doesn't pay the slow-start (runs under phase 1)
        wz = vst_pool.tile([N_CORES, 1024], BF16, name="warm_z")
        nc.vector.memset(wz[:], 0.0)
        nc.scalar.dma_start(out=warm_in[:], in_=wz[:])
        nc.gpsimd.collective_compute(
            "AllToAll", AluOpType.bypass, replica_groups=rg,
            ins=[warm_in[:].opt()], outs=[warm_out[:].opt()])

        # ================= phase 1: QKV projections + RoPE =================
        # q: one PSUM tile per head (h-outer MM loop + per-head rope drain
        # right after that head's MMs) so banks free progressively and the
        # last tb's drains never gate phase 2a's PSUM pools.
        with tc.tile_pool(name="qpsum", bufs=4, space="PSUM") as q_psum, \
             tc.tile_pool(name="kpsum", bufs=1, space="PSUM") as k_psum, \
             tc.tile_pool(name="vpsum", bufs=1, space="PSUM") as v_psum, \
             tc.tile_pool(name="vtpsum", bufs=1, space="PSUM") as vt_psum:

            def rope_drain(ps, dst):
                """dst = ps*C + shuffle16(ps)*S (tables sliced at t0)."""
                sw = rtmp_pool.tile([128, TW], F32, name="rope_sw")
                t1 = rtmp_pool.tile([128, TW], F32, name="rope_t1")
                t2 = rtmp_pool.tile([128, TW], F32, name="rope_t2")
                nc.vector.stream_shuffle(sw[:], ps, SWAP16)
                nc.vector.tensor_tensor(t1[:], sw[:], Sx, AluOpType.mult)
                nc.vector.tensor_tensor(t2[:], ps, Cx, AluOpType.mult)
                nc.vector.tensor_tensor(dst, t1[:], t2[:], AluOpType.add)

            for tb in range(cfg.NT):
                t0 = (tb % cfg.NB) * TW  # position within batch
                Cx = ropeC[:, t0:t0 + TW]
                Sx = ropeS[:, t0:t0 + TW]

                k_ps = k_psum.tile([128, TW], F32, name="k_ps")
                vT_ps = v_psum.tile([128, TW], F32, name="vT_ps")
                xts = []
                XB = 2 if tb < 2 else 4  # dc-chunks per DMA (small first)
                for ci, dc in enumerate(range(0, cfg.DC, XB)):
                    d1 = min(dc + XB, cfg.DC)
                    xt = x_pool.tile([128, (d1 - dc) * TW], BF16, name="x_t")
                    nc.sync.dma_start(
                        out=xt[:].rearrange("p (dc t) -> p dc t", dc=d1 - dc),
                        in_=xT_d.ap()[dc * 128:d1 * 128,
                                      tb * TW:(tb + 1) * TW]
                        .rearrange("(dc p) t -> p dc t", p=128))
                    for j in range(d1 - dc):
                        xts.append(xt[:, j * TW:(j + 1) * TW])
                for dc in range(cfg.DC):
                    st = dict(start=(dc == 0), stop=(dc == cfg.DC - 1))
                    nc.tensor.matmul(k_ps[:],
                                     wk_sb[:, dc * HD:(dc + 1) * HD],
                                     xts[dc], **st)
                    nc.tensor.matmul(vT_ps[:],
                                     wv_sb[:, dc * HD:(dc + 1) * HD],
                                     xts[dc], **st)
                # k: rope drain on DVE runs under the q MMs below
                rope_drain(k_ps[:], kT_sb[:, tb * TW:(tb + 1) * TW])
                vt_sb = vst_pool.tile([128, TW], BF16, name="vT_stage")
                nc.scalar.copy(vt_sb[:], vT_ps[:])
                for h in range(NHL):
                    q_ps = q_psum.tile([128, TW], F32, name="q_ps")
                    for dc in range(cfg.DC):
                        nc.tensor.matmul(
                            q_ps[:],
                            wq_sb[:, (h * cfg.DC + dc) * HD:
                                  (h * cfg.DC + dc + 1) * HD],
                            xts[dc], start=(dc == 0), stop=(dc == cfg.DC - 1))
                    rope_drain(q_ps[:],
                               qT_sb[:, h * cfg.BL + tb * TW:
                                     h * cfg.BL + (tb + 1) * TW])
                    if h == 0:
                        # v transposes tucked after h0's MMs: the vT stage
                        # copy has certainly landed by then
                        for i in range(TW // 128):
                            vp = vt_psum.tile([128, 128], BF16, name="v_tr_ps")
                            nc.tensor.transpose(vp[:],
                                                vt_sb[:, i * 128:(i + 1) * 128],
                                                ident[:])
                            nc.scalar.copy(
                                v_sb[:, tb * TW + i * 128:
                                     tb * TW + (i + 1) * 128],
                                vp[:])

        w_pool.release()
        x_pool.release()
        vst_pool.release()
        rtmp_pool.release()
        rtbl_pool.release()

        # ============ phase 2a: attention + a2a staging ====================
        # attnS_sb[c]: DMA-landing tile for gathered chunk c. The wo
        # matmuls read attnT_sb[c], produced by a DVE copy from attnS:
        # DMA-completion semaphores on the dynamic queues resolve
        # unpredictably late (sub-queue rotation / SWDGE drain), engine
        # semaphores are exact.
        at2_pool = tc.alloc_tile_pool(name="attnT", bufs=1)
        attnS_sb = [at2_pool.tile([128, KC * cfg.SH], BF16, name=f"attnS{c}")
                    for c in range(cfg.NCH)]
        attnT_sb = [at2_pool.tile([128, KC * cfg.SH], BF16, name=f"attnT{c}")
                    for c in range(cfg.NCH)]
        # wo weight stream: one n-block resident + one loading; preload
        # n=0 now so phase 2b starts without a DMA stall. Loads ride the
        # gpsimd SWDGE, chunked 4 kc per DMA: a single big DIRECT2D on a
        # compute engine stream would stall it for ~10us.
        wo_pool = tc.alloc_tile_pool(name="wo_w", bufs=2)
        woSB_t = {}

        def load_wo_nblock(n, eng):
            # ONE DMA per n-block (not chunked): a single completion
            # semaphore means the wo ldweights carries the attnT wait and
            # the matmul the woSB wait -- no event-semaphore, whose
            # wait-merging otherwise hoists late-a2a waits to the front
            # of the wo phase. n0 rides sync (before any staging); the
            # rest ride scalar, issued in phase 2b after all the exps.
            # (SWDGE mis-executes this large strided descriptor -> NaNs.)
            woSB = wo_pool.tile([128, KC * TW], BF16, name="wo_nb")
            eng.dma_start(
                out=woSB[:].rearrange("p (kc n) -> p kc n", kc=KC),
                in_=woT_d.ap()[:, n * TW:(n + 1) * TW]
                .rearrange("(kc p) n -> p kc n", p=128))
            woSB_t[n] = woSB

        load_wo_nblock(0, nc.sync)

        # one pool scope for attention + wo: a scope boundary would
        # barrier phase 2b behind the last a2a staging DMA. Attention
        # PSUM pools sit on the right side (banks freed earliest by
        # phase 1); o_psum on the left.
        with tc.tile_pool(name="expsb", bufs=4) as e_pool, \
             tc.tile_pool(name="attnsb", bufs=2) as at_pool, \
             tc.tile_pool(name="recsb", bufs=2) as rec_pool, \
             tc.tile_pool(name="ostage", bufs=4) as oc_pool, \
             tc.tile_pool(name="scps", bufs=3, space="PSUM",
                          side="right") as sc_psum, \
             tc.tile_pool(name="avps", bufs=2, space="PSUM",
                          side="right") as av_psum, \
             tc.tile_pool(name="seps", bufs=1, space="PSUM",
                          side="right") as se_psum, \
             tc.tile_pool(name="ops", bufs=2, space="PSUM") as o_psum:

            for qb in range(cfg.NB):
                active = [kt for kt in range(cfg.KT)
                          if cls[kt][qb][0] != 'N']
                offs = {kt: cls[kt][qb][1] for kt in active}
                offs[active[0]] = 0

                for b in range(cfg.B):
                    attn_sb = at_pool.tile([128, NHL * TW], BF16, name="at_sb")
                    tb2 = b * cfg.NB + qb
                    # flattened (head, key-tile) sequence with the score
                    # lookahead crossing head boundaries, so the pipeline
                    # never restarts at a head start
                    LOOKAHEAD = 2
                    n_act = len(active)
                    seq = [(h, j) for h in range(NHL) for j in range(n_act)]
                    sc_tiles = {}

                    def qt_of(h):
                        return qT_sb[:, h * cfg.BL + tb2 * TW:
                                     h * cfg.BL + (tb2 + 1) * TW]

                    def emit_sc(h, j):
                        kt2 = active[j]
                        gk2 = b * cfg.L + kt2 * KW
                        o = offs[kt2]
                        sc = sc_psum.tile([KW, TW], F32, name="sc_ps")
                        nc.tensor.matmul(sc[:, o:], kT_sb[:, gk2:gk2 + KW],
                                         qt_of(h)[:, o:],
                                         start=True, stop=True)
                        sc_tiles[(h, j)] = sc

                    for p in range(min(LOOKAHEAD, len(seq))):
                        emit_sc(*seq[p])
                    head_state = {}
                    for pos, (h, idx) in enumerate(seq):
                        if pos + LOOKAHEAD < len(seq):
                            emit_sc(*seq[pos + LOOKAHEAD])
                        if idx == 0:
                            head_state[h] = (
                                av_psum.tile([HD, TW], F32, name="at_ps"),
                                e_pool.tile([KW, TW], BF16, name="esum",
                                            bufs=2))
                        at_ps, esum = head_state[h]
                        kt = active[idx]
                        gk = b * cfg.L + kt * KW  # global key token
                        o = offs[kt]
                        sc_ps = sc_tiles.pop((h, idx))
                        ex = e_pool.tile([KW, TW], BF16, name="ex_t")
                        nc.scalar.activation(
                            ex[:, o:], sc_ps[:, o:],
                            mybir.ActivationFunctionType.Exp,
                            scale=float(SCALE))
                        if cls[kt][qb][0] == 'M':
                            # zero the masked staircase, which for
                            # 128-aligned tiles is exactly the 128
                            # query columns starting at w0: key p is
                            # kept on window column u iff u >= p.
                            w0 = kt * KW - qb * TW
                            nc.vector.tensor_tensor(
                                ex[:, w0:w0 + KW], ex[:, w0:w0 + KW],
                                tri_sb[:], AluOpType.mult)
                        # denominator: accumulate exp tiles on the DVE;
                        # one ones-matmul at the end broadcasts the sum
                        if idx == 0:
                            nc.vector.tensor_copy(esum[:], ex[:])
                        else:
                            nc.vector.tensor_tensor(
                                esum[:, o:], esum[:, o:], ex[:, o:],
                                AluOpType.add)
                        st = dict(start=(idx == 0), stop=(idx == n_act - 1))
                        nc.tensor.matmul(at_ps[:, o:], v_sb[:, gk:gk + KW],
                                         ex[:, o:], **st)
                        if idx == n_act - 1:  # head epilogue
                            se_ps = se_psum.tile([128, TW], F32, name="se_ps")
                            nc.tensor.matmul(se_ps[:], ones_sb[:], esum[:],
                                             start=True, stop=True)
                            rec = rec_pool.tile([128, TW], F32, name="rec_t")
                            nc.vector.reciprocal_approx_fast(rec[:], se_ps[:])
                            # attn_sb columns are (j, h, t): j = 128-token
                            # group, so the a2a staging DMA is one
                            # contiguous 3-dim copy per block
                            nc.vector.tensor_tensor(
                                attn_sb[:]
                                .rearrange("p (j r) -> p j r", j=TW // 128)
                                [:, :, h * 128:(h + 1) * 128],
                                at_ps[:].rearrange("p (j t) -> p j t",
                                                   j=TW // 128),
                                rec[:].rearrange("p (j t) -> p j t",
                                                 j=TW // 128),
                                AluOpType.mult)
                            del head_state[h]

                    # stage this block into a2a_in[qb]: shard (b*4+jj) rows
                    # [h*128+d] for tokens jj*128..  of this block
                    nc.sync.dma_start(
                        out=a2a_in[qb][b * (TW // 128) * QD:
                                       (b + 1) * (TW // 128) * QD, :]
                        .rearrange("(r p) t -> p r t", p=128),
                        in_=attn_sb[:].rearrange("p (r t) -> p r t",
                                                 r=(TW // 128) * NHL))
                    if b == cfg.B - 1:
                        nc.gpsimd.collective_compute(
                            "AllToAll", AluOpType.bypass, replica_groups=rg,
                            ins=[a2a_in[qb][:].opt()],
                            outs=[a2a_out[qb][:].opt()])
                        # land gathered chunk into SBUF for the wo phase.
                        # The LAST chunk rides scalar, alone: value-based
                        # DMA waits can misbind within a queue, and only
                        # this chunk's completion is late (a2a3) -- keep
                        # it off the queue everyone else waits on.
                        ceng = nc.scalar if qb == cfg.NCH - 1 else nc.sync
                        ceng.dma_start(
                            out=attnS_sb[qb][:]
                            .rearrange("p (kc t) -> p kc t", kc=KC),
                            in_=a2a_out[qb][:]
                            .rearrange("(kc p) t -> p kc t", p=128))

            # ---- phase 2b: wo = attnT.T @ woT, token-sliced ---------------
            # launder the gathered chunks through the DVE: wo matmuls then
            # wait on exact Vector-engine semaphore values
            for c in range(cfg.NCH):
                nc.vector.tensor_copy(attnT_sb[c][:], attnS_sb[c][:])

            # The LAST chunk's MM group for each n-block is deferred by one
            # n-stage: the compiler's wait-merging otherwise hoists the
            # "attnT[last] loaded" (= last a2a done) wait to the front of
            # the whole wo phase, stalling the PE ~30us.
            def wo_group(n, c, woSB):
                o_ps = o_psum.tile([128, TW], F32, name="o_ps")
                for kc in range(KC):
                    nc.tensor.matmul(
                        o_ps[:],
                        attnT_sb[c][:, kc * cfg.SH:(kc + 1) * cfg.SH],
                        woSB[:, kc * TW:(kc + 1) * TW],
                        start=(kc == 0), stop=(kc == KC - 1))
                oc = oc_pool.tile([128, TW], BF16, name="oc_t")
                if c % 2 == 0:
                    nc.vector.tensor_copy(oc[:], o_ps[:])
                else:
                    nc.scalar.copy(oc[:], o_ps[:])
                nc.sync.dma_start(
                    out=out_d.ap()[c * cfg.SH:(c + 1) * cfg.SH,
                                   n * TW:(n + 1) * TW],
                    in_=oc[:])

            cl = cfg.NCH - 1  # the a2a3-gated chunk
            for n in range(cfg.NBLK):
                if n >= 1:
                    # previous stage's deferred chunk first: frees the wo
                    # slot the upcoming load reuses
                    wo_group(n - 1, cl, woSB_t.pop(n - 1))
                if n + 1 < cfg.NBLK:
                    load_wo_nblock(n + 1, nc.sync)
                for c in range(cl):
                    wo_group(n, c, woSB_t[n])
            wo_group(cfg.NBLK - 1, cl, woSB_t.pop(cfg.NBLK - 1))

        wo_pool.release()
        at2_pool.release()
        dram_pool.release()
        q_pool.release()
        kv_pool.release()
        const_pool.release()

    nc.compile()
    return nc


def host_prepare(cfg, x, mask, wq, wk, wv, wo):
    """Returns (in_maps, cls)."""
    x = np.ascontiguousarray(np.asarray(x, dtype=np.float32))
    mask = np.asarray(mask, dtype=np.float32)
    wq = np.asarray(wq, dtype=np.float32)
    wk = np.asarray(wk, dtype=np.float32)
    wv = np.asarray(wv, dtype=np.float32)
    wo = np.asarray(wo, dtype=np.float32)

    import ml_dtypes
    bf16 = ml_dtypes.bfloat16
    perm = _rope_perm()
    C, S = _rope_tables(cfg)
    xT = np.ascontiguousarray(x.reshape(cfg.BL, cfg.D).T).astype(bf16)
    cls = classify_mask(mask, cfg)
    # full wo, transposed: row (4j+h)*128+d matches the a2a row order
    woT = np.ascontiguousarray(wo.T).astype(bf16)

    in_maps = []
    for g in range(N_CORES):
        qrows = wq[g * NHL * HD:(g + 1) * NHL * HD]          # [512, D]
        qperm = np.concatenate(
            [qrows[h * HD + perm] for h in range(NHL)], axis=0)
        krows = wk[g * HD:(g + 1) * HD][perm]                # [128, D]
        vrows = wv[g * HD:(g + 1) * HD]                      # [128, D]
        in_maps.append({
            "xT": xT,
            "wqT": np.ascontiguousarray(qperm.T).astype(bf16),
            "wkT": np.ascontiguousarray(krows.T).astype(bf16),
            "wvT": np.ascontiguousarray(vrows.T).astype(bf16),
            "woT": woT,
            "ropeC": C,
            "ropeS": S,
        })
    return in_maps, cls


def assemble_output(cfg, results):
    """Stitch per-core token slices back into [B, L, D].
    Core g's out row c*128+p = batch (g<4 ? 0 : 1), token qb=c*TW +
    (g%4)*128 + p."""
    full = np.empty((cfg.BL, cfg.D), dtype=np.float32)
    for g in range(N_CORES):
        r = np.asarray(results[g]["out"]).astype(np.float32)
        b = g // (TW // 128)
        jj = g % (TW // 128)
        for c in range(cfg.NCH):
            tok = b * cfg.L + c * TW + jj * 128
            full[tok:tok + 128] = r[c * cfg.SH:(c + 1) * cfg.SH]
    return full.reshape(cfg.B, cfg.L, cfg.D)


def kernel(x, mask, wq, wk, wv, wo):
    global LAST_RESULTS
    from concourse.bass_utils import run_bass_kernel_spmd
    cfg = Cfg(B=2, L=2048, D=4096)
    in_maps, cls = host_prepare(cfg, x, mask, wq, wk, wv, wo)
    nc = build_bass(cfg, cls)
    res = run_bass_kernel_spmd(nc, in_maps, core_ids=list(range(N_CORES)),
                               trace=TRACE)
    LAST_RESULTS = res
    return assemble_output(cfg, res.results)
